# revision 21
# baseline (speedup 1.0000x reference)
"""CrossPixContrastive loss on 8 trn2 NeuronCores.

Math (per batch n, HW=4096, C=256):
  rgb_n = l2norm_C(rgb); ir_n = l2norm_C(ir)
  e[p,q] = exp(20 * clip(<rgb_n[:,p], ir_n[:,q]>, -1, 1))
  S[p] = sum_q e ; M[p] = sum_q e * (rm_p == im_q)
  C[q] = sum_p e ; Mc[q] = sum_p e * (rm_p == im_q)
  r_rgb = M/(S+1e-6) ; r_ir = Mc/(C+1e-6)
  loss = mean(-log over nonzero of concat(r_rgb, r_ir) * fg)

Sharding: 8 cores = 4 batches x 2 halves of the rgb-pixel axis p.

The l2 normalization and fp8 quantization happen ON THE HOST:
rgb_f8 = fp8(20*rgb/||rgb_p||), ir_f8 = fp8(16*ir/||ir_q||), so the
device kernel is a pure streaming loop with no prologue compute --
the exp uses a constant 1/16 scale (recovers exp(20*cos)).

Per-core tiling: [128p x 2048q] pair-tiles (two 1024 qb chunks of the
same p-tile processed back to back):
  PE  : fp8(e4m3) DoubleRow matmuls (K=256 folded into 2 c-chunks)
        for the logits; bf16 one-hot column-sum matmuls -> C/Mc psum
        (two psC chunks live at once, one per qb of the pair)
  ACT : e = Exp(pl/16) -> bf16 with row-accum -> S (2 exps/pair)
  DVE : ONE 2048-wide masked accum (im==rm)*e -> M per pair (stt has
        no fast modes; amortize its per-instruction overhead)
PSUM: pl 2x[128,1024] (4 banks) + psC 2x[6,1024] (4 banks) = 8.
DMA issue is spread across sync/scalar/gpsimd so the input load is
not serialized on one sequencer; first-tile deps are issued first.
Host combines the tiny per-core partials into the scalar loss.
"""
import numpy as np
import ml_dtypes

import concourse.bacc as bacc
import concourse.tile as tile
from concourse import mybir
from concourse import bass_isa
from concourse.bass_utils import run_bass_kernel_spmd

dt = mybir.dt
AF = mybir.ActivationFunctionType
ALU = mybir.AluOpType
DR = mybir.MatmulPerfMode.DoubleRow

N, C, H, W = 4, 256, 64, 64
HW = H * W                      # 4096
PH = HW // 2                    # 2048  p-half per core
NPT = PH // 128                 # 16    p-tiles
QB = 1024                       # q chunk (psum tile width)
NQB = HW // QB                  # 4
NQP = NQB // 2                  # 2     qb pairs
NCLS = 5
EXP_SCALE = 1.0 / 16.0          # recovers exp(20*cos)
EPS_DEN = 1e-6
DEFER = 2                       # col-matmul deferral (pairs)

_CACHED_NC = None


def build_nc():
    nc = bacc.Bacc("TRN2", target_bir_lowering=False, debug=False, num_devices=8)

    rgb_in = nc.dram_tensor("rgb_f8", [2, 128, PH], dt.float8e4, kind="ExternalInput").ap()
    ir_in = nc.dram_tensor("ir_f8", [2, 128, HW], dt.float8e4, kind="ExternalInput").ap()
    im_in = nc.dram_tensor("im_bcast", [128, HW], dt.float8e4, kind="ExternalInput").ap()
    rm_in = nc.dram_tensor("rm_cols", [128, NPT], dt.bfloat16, kind="ExternalInput").ap()
    oh_in = nc.dram_tensor("oh_lhsT", [128, NPT * 6], dt.bfloat16, kind="ExternalInput").ap()

    S_out = nc.dram_tensor("S_out", [128, NPT * NQB], dt.float32, kind="ExternalOutput").ap()
    # one extra slot: the warmup pair's first half writes slot NPT*NQP
    M_out = nc.dram_tensor("M_out", [128, NPT * NQP + 1], dt.float32, kind="ExternalOutput").ap()
    C_out = nc.dram_tensor("C_out", [6, HW], dt.float32, kind="ExternalOutput").ap()

    with tile.TileContext(nc) as tc:
        with tc.tile_pool(name="big", bufs=1) as big, \
             tc.tile_pool(name="epool", bufs=4) as epool, \
             tc.tile_pool(name="psL", bufs=2, space="PSUM") as psL, \
             tc.tile_pool(name="psC", bufs=2, space="PSUM") as psCp:

            # ---------------- persistent tiles ----------------
            im_b = big.tile([128, HW], dt.float8e4)
            rm_c = big.tile([128, NPT], dt.bfloat16)
            oh_b = big.tile([128, NPT * 6], dt.bfloat16)
            rgb_f8 = big.tile([128, 2, PH], dt.float8e4)
            ir_f8 = big.tile([128, 2, HW], dt.float8e4)
            S_stat = big.tile([128, NPT * NQB], dt.float32)
            M_stat = big.tile([128, NPT * NQP + 1], dt.float32)
            C_sb = big.tile([6, HW], dt.float32)
            junk = big.tile([128, 2 * QB], dt.float8e5)

            # ---------------- input DMAs ----------------
            # Parallel issue across the three DMA-capable sequencers;
            # everything pair-0 needs (rgb, ir[0:2048] both c-chunks,
            # im[0:2048], rm) is issued first in 64KB chunks, the rest
            # streams in under the main loop.
            def S(a, b):
                return slice(a, b)
            # scalar: ir qb0 chunks + rm + im[0:1024] + oh
            nc.scalar.dma_start(ir_f8[:, 0, S(0, 512)], ir_in[0, :, S(0, 512)])
            nc.scalar.dma_start(ir_f8[:, 1, S(0, 512)], ir_in[1, :, S(0, 512)])
            nc.scalar.dma_start(ir_f8[:, 0, S(512, 1024)], ir_in[0, :, S(512, 1024)])
            nc.scalar.dma_start(ir_f8[:, 1, S(512, 1024)], ir_in[1, :, S(512, 1024)])
            nc.scalar.dma_start(rm_c[:], rm_in)
            nc.scalar.dma_start(im_b[:, S(0, 512)], im_in[:, S(0, 512)])
            nc.scalar.dma_start(im_b[:, S(512, 1024)], im_in[:, S(512, 1024)])
            nc.scalar.dma_start(oh_b[:], oh_in)
            # sync: rgb c0 + ir qb1 c0 + im[1024:1536] + rest c0
            for h in range(4):
                cs = S(h * 512, (h + 1) * 512)
                nc.sync.dma_start(rgb_f8[:, 0, cs], rgb_in[0, :, cs])
            nc.sync.dma_start(ir_f8[:, 0, S(1024, 1536)], ir_in[0, :, S(1024, 1536)])
            nc.sync.dma_start(ir_f8[:, 0, S(1536, 2048)], ir_in[0, :, S(1536, 2048)])
            nc.sync.dma_start(im_b[:, S(1024, 1536)], im_in[:, S(1024, 1536)])
            nc.sync.dma_start(ir_f8[:, 0, S(2048, HW)], ir_in[0, :, S(2048, HW)])
            nc.sync.dma_start(im_b[:, S(2048, 3072)], im_in[:, S(2048, 3072)])
            # gpsimd: rgb c1 + ir qb1 c1 + im[1536:2048] + rest c1
            for h in range(4):
                cs = S(h * 512, (h + 1) * 512)
                nc.gpsimd.dma_start(rgb_f8[:, 1, cs], rgb_in[1, :, cs])
            nc.gpsimd.dma_start(ir_f8[:, 1, S(1024, 1536)], ir_in[1, :, S(1024, 1536)])
            nc.gpsimd.dma_start(ir_f8[:, 1, S(1536, 2048)], ir_in[1, :, S(1536, 2048)])
            nc.gpsimd.dma_start(im_b[:, S(1536, 2048)], im_in[:, S(1536, 2048)])
            nc.gpsimd.dma_start(ir_f8[:, 1, S(2048, HW)], ir_in[1, :, S(2048, HW)])
            nc.gpsimd.dma_start(im_b[:, S(3072, HW)], im_in[:, S(3072, HW)])

            # ---------------- main loop ----------------
            pending = []
            psC_cur = [None, None]

            def flush_one():
                e_pair, qp0, pt0 = pending.pop(0)
                if pt0 == 0:
                    for h in range(2):
                        psC_cur[h] = psCp.tile([6, QB], dt.float32, tag="psC",
                                               name=f"psC{2 * qp0 + h}")
                for h in range(2):
                    psCq = psC_cur[h]
                    for half in range(2):
                        nc.tensor.matmul(
                            psCq[:, half * 512:(half + 1) * 512],
                            oh_b[:, pt0 * 6:(pt0 + 1) * 6],
                            e_pair[:, h * QB + half * 512:h * QB + (half + 1) * 512],
                            start=(pt0 == 0), stop=(pt0 == NPT - 1))
                if pt0 == NPT - 1:
                    for h in range(2):
                        qb0 = 2 * qp0 + h
                        # split psum->SBUF copy across both engines
                        cs0 = slice(qb0 * QB, qb0 * QB + 512)
                        cs1 = slice(qb0 * QB + 512, (qb0 + 1) * QB)
                        nc.vector.tensor_copy(C_sb[:, cs0], psC_cur[h][:, 0:512])
                        nc.scalar.activation(C_sb[:, cs1], psC_cur[h][:, 512:QB],
                                             AF.Copy)
                        nc.sync.dma_start(C_out[:, qb0 * QB:(qb0 + 1) * QB],
                                          C_sb[:, qb0 * QB:(qb0 + 1) * QB])

            for qp in range(NQP):
                for pt in range(NPT):
                    po = pt * 128
                    e_pair = epool.tile([128, 2 * QB], dt.bfloat16, tag="e")
                    warm = (qp == 0 and pt == 0)
                    for h in range(2):
                        qb = 2 * qp + h
                        t = pt * NQB + qb
                        pl = psL.tile([128, QB], dt.float32, tag="pl")
                        for half in range(2):
                            qo = qb * QB + half * 512
                            nc.tensor.matmul(pl[:, half * 512:(half + 1) * 512],
                                             rgb_f8[:, :, po:po + 128],
                                             ir_f8[:, :, qo:qo + 512],
                                             start=True, stop=True, perf_mode=DR)
                        nc.scalar.activation(e_pair[:, h * QB:(h + 1) * QB],
                                             pl[:], AF.Exp,
                                             scale=EXP_SCALE,
                                             accum_out=S_stat[:, t:t + 1])
                        if warm:
                            # warmup: per-qb stt so DVE starts right
                            # after the first exp (slot 32 for h=0)
                            ws = NPT * NQP if h == 0 else 0
                            nc.vector.scalar_tensor_tensor(
                                out=junk[:, 0:QB],
                                in0=im_b[:, qb * QB:(qb + 1) * QB],
                                scalar=rm_c[:, pt:pt + 1],
                                in1=e_pair[:, h * QB:(h + 1) * QB],
                                op0=ALU.is_equal, op1=ALU.mult,
                                accum_out=M_stat[:, ws:ws + 1])
                    if not warm:
                        ts = pt * NQP + qp
                        nc.vector.scalar_tensor_tensor(
                            out=junk[:],
                            in0=im_b[:, qp * 2 * QB:(qp + 1) * 2 * QB],
                            scalar=rm_c[:, pt:pt + 1],
                            in1=e_pair[:],
                            op0=ALU.is_equal, op1=ALU.mult,
                            accum_out=M_stat[:, ts:ts + 1])
                    pending.append((e_pair, qp, pt))
                    if len(pending) > DEFER:
                        flush_one()
            while pending:
                flush_one()

            # ---------------- outputs (host reduces the stats) --------
            nc.sync.dma_start(S_out, S_stat[:])
            nc.gpsimd.dma_start(M_out, M_stat[:])

    nc.compile()
    return nc


def _get_nc():
    global _CACHED_NC
    if _CACHED_NC is None:
        _CACHED_NC = build_nc()
    return _CACHED_NC


def _build_in_maps(np_inputs):
    f32 = np.float32
    rgb_map = np.asarray(np_inputs["rgb_map"], dtype=f32).reshape(N, C, HW)
    ir_map = np.asarray(np_inputs["ir_map"], dtype=f32).reshape(N, C, HW)
    rm = np.asarray(np_inputs["rgb_mask"]).reshape(N, HW)
    im = np.asarray(np_inputs["ir_mask"]).reshape(N, HW)
    rm_f = rm.astype(f32)
    im_f8 = im.astype(ml_dtypes.float8_e4m3fn)

    # host-side l2 normalization + fp8 quantization (carry 20x / 16x)
    rn = np.sqrt(np.sum(rgb_map * rgb_map, axis=1, keepdims=True))
    rgb_n = rgb_map * (20.0 / np.maximum(rn, 1e-12))
    inn = np.sqrt(np.sum(ir_map * ir_map, axis=1, keepdims=True))
    ir_n = ir_map * (16.0 / np.maximum(inn, 1e-12))
    rgb_q = rgb_n.astype(ml_dtypes.float8_e4m3fn)   # (N, C, HW)
    ir_q = ir_n.astype(ml_dtypes.float8_e4m3fn)

    in_maps = []
    for core in range(8):
        n, h = core // 2, core % 2
        psl = slice(h * PH, (h + 1) * PH)
        rgb_f8 = np.ascontiguousarray(rgb_q[n, :, psl].reshape(2, 128, PH))
        ir_f8 = np.ascontiguousarray(ir_q[n].reshape(2, 128, HW))
        im_bc = np.broadcast_to(im_f8[n], (128, HW)).copy()
        rm_half = rm_f[n, psl]
        rm_cols = np.ascontiguousarray(rm_half.reshape(NPT, 128).T).astype(
            ml_dtypes.bfloat16)
        oh = np.empty((NPT, 128, 6), dtype=f32)
        oh[:, :, 0] = 1.0
        rm_tiles = rm_half.reshape(NPT, 128)
        for k in range(NCLS):
            oh[:, :, 1 + k] = (rm_tiles == k)
        oh_lhsT = np.ascontiguousarray(
            oh.transpose(1, 0, 2).reshape(128, NPT * 6)).astype(ml_dtypes.bfloat16)
        in_maps.append({
            "rgb_f8": rgb_f8,
            "ir_f8": ir_f8,
            "im_bcast": im_bc,
            "rm_cols": rm_cols,
            "oh_lhsT": oh_lhsT,
        })
    return in_maps


def _stat_to_p(arr, ncol):
    """[128, NPT*ncol] per-tile stats -> [PH] per-pixel sums (f64)."""
    return arr.astype(np.float64).reshape(128, NPT, ncol).sum(2).T.reshape(PH)


def kernel(rgb_map, ir_map, rgb_mask, ir_mask):
    np_inputs = {"rgb_map": rgb_map, "ir_map": ir_map,
                 "rgb_mask": rgb_mask, "ir_mask": ir_mask}
    in_maps = _build_in_maps(np_inputs)
    im = np.asarray(ir_mask).reshape(N, HW)
    rm = np.asarray(rgb_mask).reshape(N, HW)

    nc = _get_nc()
    res = run_bass_kernel_spmd(nc, in_maps, list(range(8)))

    # ---------------- host combine (tiny) ----------------
    entries = []
    for n in range(N):
        rA, rB = res.results[2 * n], res.results[2 * n + 1]
        S = np.concatenate([_stat_to_p(rA["S_out"], NQB), _stat_to_p(rB["S_out"], NQB)])

        def _m_to_p(r):
            m = r["M_out"].astype(np.float64)
            m[:, 0] += m[:, NPT * NQP]          # warmup half-slot
            return _stat_to_p(m[:, :NPT * NQP], NQP)
        M = np.concatenate([_m_to_p(rA), _m_to_p(rB)])
        C6 = rA["C_out"].astype(np.float64) + rB["C_out"].astype(np.float64)
        Ce = C6[0]
        imn = im[n]
        Mc = C6[1 + imn, np.arange(HW)]
        r_rgb = (M / (S + EPS_DEN)) * (rm[n] > 0)
        r_ir = (Mc / (Ce + EPS_DEN)) * (imn > 0)
        entries.append(r_rgb)
        entries.append(r_ir)
    L = np.concatenate(entries)
    nz = L != 0
    total = -np.log(L[nz]).sum() if nz.any() else 0.0
    count = max(float(nz.sum()), 1.0)
    return np.asarray(np.float32(total / count))


if __name__ == "__main__":
    import reference
    inputs = reference.setup_inputs()
    inputs = {k: np.asarray(v) for k, v in inputs.items()}
    out = kernel(**inputs)
    print("kernel:", out)
